# Initial kernel scaffold
#
"""CZ-ring diagonal sign kernel for Trainium2 (8 NeuronCores).

Math: out = sign[row] * (x_real + 1j * x_imag), where sign is the ±1
diagonal of a CZ ring circuit on 13 qubits (computable from the row index).

Sharding: rows (the 2^13 = 8192 state dim) split across 8 cores, 1024 rows
each — contiguous zero-copy slices of the inputs and of the complex64
output. The 8192-entry sign vector is computed on host (tiny) and each
core gets its 1024-entry slice, pre-transposed to [128 partitions x 8
row-tiles].

On-chip per core: for each of 8 row-tiles [128, 4096], load x_real/x_imag,
multiply by the per-partition sign (real product on the vector engine,
imag product on the scalar engine), writing into an interleaved
[128, 4096, 2] tile that has exactly the complex64 memory layout, then
store contiguously. Memory-bound: 64 MiB HBM traffic per core.
"""

import sys

for _p in ("/opt/trn_rl_repo", "/root/.axon_site/_ro/trn_rl_repo"):
    if _p not in sys.path:
        sys.path.append(_p)

import numpy as np

N_WIRES = 13
DIM = 2**N_WIRES  # 8192
BATCH = 4096
N_CORES = 8
ROWS_PER_CORE = DIM // N_CORES  # 1024
P = 128
N_ROW_TILES = ROWS_PER_CORE // P  # 8


def _cz_ring_signs() -> np.ndarray:
    idx = np.arange(DIM, dtype=np.int64)
    shifts = N_WIRES - 1 - np.arange(N_WIRES)
    bits = (idx[:, None] >> shifts[None, :]) & 1
    parity = (bits[:, :-1] * bits[:, 1:]).sum(axis=1) + bits[:, 0] * bits[:, -1]
    return np.where(parity % 2 == 1, -1.0, 1.0).astype(np.float32)


_SIGN = _cz_ring_signs()  # [8192]

_NC = None


def _build_module():
    global _NC
    if _NC is not None:
        return _NC

    import concourse.bacc as bacc
    import concourse.tile as tile
    from concourse import mybir

    nc = bacc.Bacc("TRN2", target_bir_lowering=False, debug=False,
                   num_devices=N_CORES)
    xr = nc.dram_tensor("x_real", [ROWS_PER_CORE, BATCH], mybir.dt.float32,
                        kind="ExternalInput").ap()
    xi = nc.dram_tensor("x_imag", [ROWS_PER_CORE, BATCH], mybir.dt.float32,
                        kind="ExternalInput").ap()
    sg = nc.dram_tensor("sign", [P, N_ROW_TILES], mybir.dt.float32,
                        kind="ExternalInput").ap()
    out = nc.dram_tensor("out", [ROWS_PER_CORE, BATCH, 2], mybir.dt.float32,
                         kind="ExternalOutput").ap()

    with tile.TileContext(nc) as tc:
        with tc.tile_pool(name="sign", bufs=1) as sign_pool, \
             tc.tile_pool(name="inp", bufs=2) as in_pool, \
             tc.tile_pool(name="outp", bufs=2) as out_pool:
            sign_sb = sign_pool.tile([P, N_ROW_TILES], mybir.dt.float32)
            nc.sync.dma_start(out=sign_sb[:], in_=sg[:])
            for t in range(N_ROW_TILES):
                rows = slice(t * P, (t + 1) * P)
                xr_t = in_pool.tile([P, BATCH], mybir.dt.float32, tag="xr")
                nc.sync.dma_start(out=xr_t[:], in_=xr[rows, :])
                xi_t = in_pool.tile([P, BATCH], mybir.dt.float32, tag="xi")
                nc.sync.dma_start(out=xi_t[:], in_=xi[rows, :])
                o_t = out_pool.tile([P, BATCH, 2], mybir.dt.float32, tag="o")
                s_t = sign_sb[:, t:t + 1]
                nc.vector.tensor_scalar_mul(o_t[:, :, 0], xr_t[:], s_t)
                nc.scalar.mul(o_t[:, :, 1], xi_t[:], s_t)
                nc.sync.dma_start(out=out[rows], in_=o_t[:])

    nc.compile()
    _NC = nc
    return nc


def run(x_real, x_imag, trace=False, trace_kwargs=None):
    """Run on 8 cores; returns (complex64 output, BassKernelResults)."""
    from concourse.bass_utils import run_bass_kernel_spmd

    nc = _build_module()

    x_real = np.ascontiguousarray(np.asarray(x_real), dtype=np.float32)
    x_imag = np.ascontiguousarray(np.asarray(x_imag), dtype=np.float32)
    assert x_real.shape == (DIM, BATCH) and x_imag.shape == (DIM, BATCH)

    in_maps = []
    for k in range(N_CORES):
        r0 = k * ROWS_PER_CORE
        sl = slice(r0, r0 + ROWS_PER_CORE)
        sgn_k = np.ascontiguousarray(
            _SIGN[sl].reshape(N_ROW_TILES, P).T)  # [128, 8]
        in_maps.append({
            "x_real": x_real[sl],
            "x_imag": x_imag[sl],
            "sign": sgn_k,
        })

    kw = {}
    if trace:
        kw["trace"] = True
        if trace_kwargs:
            kw["trace_kwargs"] = trace_kwargs
    res = run_bass_kernel_spmd(nc, in_maps, list(range(N_CORES)), **kw)

    full = np.empty((DIM, BATCH), dtype=np.complex64)
    fullv = full.view(np.float32).reshape(DIM, BATCH, 2)
    for k in range(N_CORES):
        r0 = k * ROWS_PER_CORE
        fullv[r0:r0 + ROWS_PER_CORE] = res.results[k]["out"]
    return full, res


def kernel(x_real, x_imag):
    out, _ = run(x_real, x_imag, trace=False)
    return out


# revision 3
# speedup vs baseline: 1.0390x; 1.0390x over previous
"""CZ-ring diagonal sign kernel for Trainium2 (8 NeuronCores).

Math: out = sign[row] * (x_real + 1j * x_imag), where sign is the ±1
diagonal of a CZ ring circuit on 13 qubits (computable from the row index).

Sharding: rows (the 2^13 = 8192 state dim) split across 8 cores, 1024 rows
each — contiguous zero-copy slices of the inputs and of the complex64
output. The 8192-entry sign vector is computed on host (tiny) and each
core gets its 1024-entry slice, pre-transposed to [128 partitions x 8
row-tiles].

On-chip per core: for each of 8 row-tiles [128, 4096], load x_real/x_imag,
multiply by the per-partition sign (real product on the vector engine,
imag product on the scalar engine), writing into an interleaved
[128, 4096, 2] tile that has exactly the complex64 memory layout, then
store contiguously. Memory-bound: 64 MiB HBM traffic per core.
"""

import sys

for _p in ("/opt/trn_rl_repo", "/root/.axon_site/_ro/trn_rl_repo"):
    if _p not in sys.path:
        sys.path.append(_p)

import numpy as np

N_WIRES = 13
DIM = 2**N_WIRES  # 8192
BATCH = 4096
N_CORES = 8
ROWS_PER_CORE = DIM // N_CORES  # 1024
P = 128
N_ROW_TILES = ROWS_PER_CORE // P  # 8


def _cz_ring_signs() -> np.ndarray:
    idx = np.arange(DIM, dtype=np.int64)
    shifts = N_WIRES - 1 - np.arange(N_WIRES)
    bits = (idx[:, None] >> shifts[None, :]) & 1
    parity = (bits[:, :-1] * bits[:, 1:]).sum(axis=1) + bits[:, 0] * bits[:, -1]
    return np.where(parity % 2 == 1, -1.0, 1.0).astype(np.float32)


_SIGN = _cz_ring_signs()  # [8192]

_NC_CACHE = {}


def _build_module(reps=1):
    """Build the per-core Bass module. `reps` repeats the full sweep
    (load -> sign-multiply -> store) back to back inside one NEFF; used
    only for benchmarking throughput (reps=1 is the real kernel)."""
    if reps in _NC_CACHE:
        return _NC_CACHE[reps]

    import concourse.bacc as bacc
    import concourse.tile as tile
    from concourse import mybir

    nc = bacc.Bacc("TRN2", target_bir_lowering=False, debug=False,
                   num_devices=N_CORES)
    xr = nc.dram_tensor("x_real", [ROWS_PER_CORE, BATCH], mybir.dt.float32,
                        kind="ExternalInput").ap()
    xi = nc.dram_tensor("x_imag", [ROWS_PER_CORE, BATCH], mybir.dt.float32,
                        kind="ExternalInput").ap()
    sg = nc.dram_tensor("sign", [P, N_ROW_TILES], mybir.dt.float32,
                        kind="ExternalInput").ap()
    out = nc.dram_tensor("out", [ROWS_PER_CORE, BATCH, 2], mybir.dt.float32,
                         kind="ExternalOutput").ap()

    with tile.TileContext(nc) as tc:
        with tc.tile_pool(name="sign", bufs=1) as sign_pool, \
             tc.tile_pool(name="inp", bufs=2) as in_pool, \
             tc.tile_pool(name="outp", bufs=2) as out_pool:
            sign_sb = sign_pool.tile([P, N_ROW_TILES], mybir.dt.float32)
            nc.sync.dma_start(out=sign_sb[:], in_=sg[:])
            for _ in range(reps):
                for t in range(N_ROW_TILES):
                    rows = slice(t * P, (t + 1) * P)
                    xr_t = in_pool.tile([P, BATCH], mybir.dt.float32, tag="xr")
                    nc.sync.dma_start(out=xr_t[:], in_=xr[rows, :])
                    xi_t = in_pool.tile([P, BATCH], mybir.dt.float32, tag="xi")
                    nc.sync.dma_start(out=xi_t[:], in_=xi[rows, :])
                    o_t = out_pool.tile([P, BATCH, 2], mybir.dt.float32,
                                        tag="o")
                    s_t = sign_sb[:, t:t + 1]
                    nc.vector.tensor_scalar_mul(o_t[:, :, 0], xr_t[:], s_t)
                    nc.scalar.mul(o_t[:, :, 1], xi_t[:], s_t)
                    nc.sync.dma_start(out=out[rows], in_=o_t[:])

    nc.compile()
    _NC_CACHE[reps] = nc
    return nc


def _make_in_maps(x_real, x_imag):
    x_real = np.ascontiguousarray(np.asarray(x_real), dtype=np.float32)
    x_imag = np.ascontiguousarray(np.asarray(x_imag), dtype=np.float32)
    assert x_real.shape == (DIM, BATCH) and x_imag.shape == (DIM, BATCH)

    in_maps = []
    for k in range(N_CORES):
        r0 = k * ROWS_PER_CORE
        sl = slice(r0, r0 + ROWS_PER_CORE)
        sgn_k = np.ascontiguousarray(
            _SIGN[sl].reshape(N_ROW_TILES, P).T)  # [128, 8]
        in_maps.append({
            "x_real": x_real[sl],
            "x_imag": x_imag[sl],
            "sign": sgn_k,
        })
    return in_maps


def run(x_real, x_imag, trace=False, trace_kwargs=None):
    """Run on 8 cores; returns (complex64 output, BassKernelResults)."""
    from concourse.bass_utils import run_bass_kernel_spmd

    nc = _build_module()
    in_maps = _make_in_maps(x_real, x_imag)

    kw = {}
    if trace:
        kw["trace"] = True
        if trace_kwargs:
            kw["trace_kwargs"] = trace_kwargs
    res = run_bass_kernel_spmd(nc, in_maps, list(range(N_CORES)), **kw)

    full = np.empty((DIM, BATCH), dtype=np.complex64)
    fullv = full.view(np.float32).reshape(DIM, BATCH, 2)
    for k in range(N_CORES):
        r0 = k * ROWS_PER_CORE
        fullv[r0:r0 + ROWS_PER_CORE] = res.results[k]["out"]
    return full, res


def kernel(x_real, x_imag):
    out, _ = run(x_real, x_imag, trace=False)
    return out
